# revision 6
# baseline (speedup 1.0000x reference)
"""Trainium2 Bass kernel for nn_CrossAttention_28183575396415.

The reference block-mask gives every query exactly one key (kv = q_idx // 3),
so the softmax weight is identically 1 and the q/k projections, RMSNorm and
RoPE are dead code.  The module reduces to

    out[b, t] = x_kv[b, t // 3] @ Wv.T @ Wproj.T
              = x_kv[b, t // 3] @ WfT          with WfT = Wv.T @ Wproj.T

Strategy (8 NeuronCores, SPMD):
  - Host folds the two projection matrices into WfT (computed in float64,
    stored float32) — constant folding of adjacent linear layers.
  - The 4*2048 = 8192 kv rows are row-sharded 8 ways (1024 rows/core).
    Each core's shard is pre-transposed on host so every device DMA is a
    natural contiguous load and the PE needs no on-device transposes; the
    shard and the weight are concatenated into one [1024(k), 2048] input so
    each k-tile arrives in a single 1 MiB DMA:
        xw[:, :1024]  = x_shard.T   (k on partitions = contraction dim)
        xw[:, 1024:]  = WfT
  - Device: z = xT.T @ WfT with K accumulated in PSUM (8 k-tiles), then each
    z row tile is written to HBM three times (the t//3 replication), giving
    this core's contiguous [3072, 1024] slice of the flattened output.
  - Host unshard = concatenate the 8 slices.
"""

import json
import os

import numpy as np

import concourse.bass as bass
import concourse.mybir as mybir
from concourse.tile import TileContext
from concourse.bass_utils import run_bass_kernel_spmd

P = 128          # partitions
C = 1024         # model dim
K_T = C // P     # k tiles
M_T = C // P     # row tiles per core shard
N = 512          # matmul free dim (one PSUM bank of fp32)
L = 3            # replication factor (Tq // Tkv)
ROWS_PER_CORE = 1024
N_CORES = 8

# compute dtype: "f32r" (full-rate fp32 PE mode), "bf16", or "f32" (4x slower)
COMPUTE_DT = os.environ.get("KERNEL_COMPUTE_DT", "f32r")
# "device3": device writes the replicated [3072, 1024] slice (default)
# "host1":   device writes [1024, 1024]; host repeats rows (debug/compare only)
OUT_MODE = os.environ.get("KERNEL_OUT_MODE", "device3")


def _split_multiwaits(nc: bass.Bass) -> None:
    """This container's walrus allows only ONE sync-wait on several
    instruction formats (Drain/CTRL, Matmult's LDWEIGHTS half, ...).  Tile
    can emit more.  Post-pass the serialized BIR: for any instruction with
    >1 on_wait, hoist all but the last wait onto single-wait EventSemaphore
    carriers inserted immediately before it on the same engine (waits then
    execute in queue order — semantics unchanged).  The patched JSON is
    pinned on the instance so every downstream serialization sees it."""
    raw = bass.Bass.to_json_bytes(nc)
    j = json.loads(raw)
    n_hoisted = 0
    for f in j["functions"]:
        for bb in f["blocks"]:
            new_insts = []
            for ins in bb["instructions"]:
                si = ins.get("sync_info")
                waits = si.get("on_wait", []) if si else []
                if len(waits) > 1:
                    for i, w in enumerate(waits[:-1]):
                        carrier = {
                            "engine": ins["engine"],
                            "ins": [],
                            "outs": [],
                            "name": f"{ins['name']}_hw{i}",
                            "opcode": "EventSemaphore",
                            "sync_info": {"on_update": [], "on_wait": [w]},
                        }
                        if "debug" in ins:
                            carrier["debug"] = ins["debug"]
                        new_insts.append(carrier)
                        n_hoisted += 1
                    si["on_wait"] = waits[-1:]
                new_insts.append(ins)
            bb["instructions"] = new_insts
    patched = json.dumps(j).encode()
    nc.to_json_bytes = lambda: patched


def _build(compute_dt: str, out_mode: str) -> bass.Bass:
    nc = bass.Bass("TRN2")
    in_mydt = {
        "bf16": mybir.dt.bfloat16,
        "f32r": mybir.dt.float32r,
        "f32": mybir.dt.float32,
    }[compute_dt]

    W2 = ROWS_PER_CORE + C  # concatenated [x | w] free dim
    xw = nc.dram_tensor("xw", [C, W2], in_mydt, kind="ExternalInput")
    n_rep = L if out_mode == "device3" else 1
    out = nc.dram_tensor(
        "out", [n_rep * ROWS_PER_CORE, C], mybir.dt.float32, kind="ExternalOutput"
    )

    xw_t = xw.rearrange("(t p) m -> t p m", p=P)  # [8, 128, 2048]
    # out row (n_rep*g + r) <- z row g
    out_rep = out.rearrange("(g r) c -> g r c", r=n_rep)  # [1024, n_rep, 1024]

    with TileContext(nc) as tc:
        with (
            tc.tile_pool(name="xw", bufs=1) as xw_pool,
            tc.tile_pool(name="psum", bufs=8, space="PSUM") as psum_pool,
            tc.tile_pool(name="zout", bufs=4) as z_pool,
        ):
            xwk = []
            for k in range(K_T):
                t = xw_pool.tile([P, W2], in_mydt, name=f"xw{k}", tag=f"xw{k}")
                nc.sync.dma_start(t[:], xw_t[k])
                xwk.append(t)

            # Two phases of 4 row-tiles so phase A only needs the first k
            # tiles to start (compute overlaps the remaining input DMAs)
            # while the 8 PSUM banks stay fully subscribed.
            for ms in (range(0, M_T // 2), range(M_T // 2, M_T)):
                pss = {}
                for m in ms:
                    pss[m] = tuple(
                        psum_pool.tile(
                            [P, N], mybir.dt.float32, name=f"ps{m}_{c}", tag="ps"
                        )
                        for c in range(2)
                    )
                for k in range(K_T):
                    for m in ms:
                        lhs = xwk[k][:, m * P : (m + 1) * P]
                        first, last = k == 0, k == K_T - 1
                        for c in range(2):
                            nc.tensor.matmul(
                                pss[m][c][:],
                                lhs,
                                xwk[k][:, ROWS_PER_CORE + c * N : ROWS_PER_CORE + (c + 1) * N],
                                start=first,
                                stop=last,
                            )
                for m in ms:
                    z = z_pool.tile([P, C], mybir.dt.float32, name=f"z{m}", tag="z")
                    for c in range(2):
                        nc.vector.tensor_copy(z[:, c * N : (c + 1) * N], pss[m][c][:])
                    for r in range(n_rep):
                        nc.sync.dma_start(out_rep[m * P : (m + 1) * P, r, :], z[:])

    _split_multiwaits(nc)
    return nc


_NC_CACHE: dict = {}


def _get_nc(compute_dt: str, out_mode: str) -> bass.Bass:
    key = (compute_dt, out_mode)
    if key not in _NC_CACHE:
        _NC_CACHE[key] = _build(compute_dt, out_mode)
    return _NC_CACHE[key]


def kernel(x_q, x_kv, Wq, Wk, Wv, Wproj):
    B, Tkv, C_ = x_kv.shape
    assert (B, Tkv, C_) == (4, 2048, C)

    # Fold the two projections: z = x @ Wv.T @ Wproj.T = x @ WfT
    WfT = (Wv.astype(np.float64).T @ Wproj.astype(np.float64).T).astype(np.float32)

    x_flat = x_kv.reshape(B * Tkv, C)
    in_maps = []
    for c in range(N_CORES):
        shard = x_flat[c * ROWS_PER_CORE : (c + 1) * ROWS_PER_CORE]
        xw = np.concatenate([shard.T, WfT], axis=1)  # [C(k), 2048]
        if COMPUTE_DT == "bf16":
            import ml_dtypes

            xw = xw.astype(ml_dtypes.bfloat16)
        else:
            xw = np.ascontiguousarray(xw)
        in_maps.append({"xw": xw})

    nc = _get_nc(COMPUTE_DT, OUT_MODE)
    res = run_bass_kernel_spmd(nc, in_maps, core_ids=list(range(N_CORES)))

    Tq = L * Tkv
    blocks = []
    for c in range(N_CORES):
        blk = res.results[c]["out"]
        if OUT_MODE != "device3":
            blk = np.repeat(blk, L, axis=0)
        blocks.append(blk)
    out_flat = np.concatenate(blocks, axis=0)  # [B*Tq, C]
    return out_flat.reshape(B, Tq, C)
